# revision 1
# baseline (speedup 1.0000x reference)
"""Trainium2 Bass kernel for CombinedLoss (focal + dice + boundary-weighted BCE).

Contract: kernel(inputs, targets) takes FULL (64,1,512,512) fp32 arrays and
returns the full scalar loss (fp32). Internally: pure data-parallel over the
batch dim -- 8 images per NeuronCore on 8 cores. Each core computes partial
sums (per-partition accumulators + PSUM dot-product tiles); the host combines
them in float64.

Math used (t in {0,1}):
  z  = (2t-1)*x           (computed as 2*(t-0.5)*x, zh=(t-0.5)*x kept in bf16)
  bce = softplus(-z)       pt = exp(-bce) = sigmoid(z)
  p = sigmoid(x) = 1 - t - pt + 2*t*pt   (only sums needed)
  focal = mean(0.25*(1-pt)^2*bce)
  dice uses  sum(p*t) = sum(t*pt),  sum(p),  sum(t)
  2-iter erosion/dilation (cross structure) == threshold of the 13-point
  diamond stencil sum W = conv2(m, cross (x) cross):  ero2 = [W >= 24.5],
  dil2 = [W >= 0.5];  boundary weights = 1 + 5*(dil2 - ero2).
  W = convh(m,[1,2,5,2,1]) + 2*convh(A,[1,1,1]) + B,  A = m(r-1)+m(r+1),
  B = m(r-2)+m(r+2)   (all integer sums <= 25: exact in bf16).

Layout per core: [128 partitions = 8 img x 16 row-blocks, 32 rows, 512 cols],
processed in 4 column strips of 128 (+2 halo cols). Row halos (+-2) cross
partitions and are filled with small SBUF->SBUF DMAs; image borders memset 0
(matches scipy border_value=0).
"""

import numpy as np

N_CORES = 8
IMG_PER_CORE = 8
H = 512
W = 512
ROWS_PB = 32          # rows per partition block
BLKS = H // ROWS_PB   # 16 blocks per image
P = 128               # partitions = IMG_PER_CORE * BLKS
STRIPS = 4
SW = W // STRIPS      # strip width = 128
N_TOTAL = 64 * 1 * H * W

_CACHE = {}


def _build():
    from concourse import bacc, mybir, tile

    f32 = mybir.dt.float32
    bf16 = mybir.dt.bfloat16
    Alu = mybir.AluOpType
    Act = mybir.ActivationFunctionType

    nc = bacc.Bacc("TRN2", target_bir_lowering=False, debug=False,
                   num_devices=N_CORES)

    x_d = nc.dram_tensor("x", [P, ROWS_PB, W], f32, kind="ExternalInput").ap()
    t_d = nc.dram_tensor("t", [P, ROWS_PB, W], f32, kind="ExternalInput").ap()
    # acc columns: [strip] = sum(t), [4+strip] = sum(pt), [8+strip] = sum(bce)
    acc_d = nc.dram_tensor("acc", [P, 16], f32, kind="ExternalOutput").ap()
    # dots: 0 = sum(m*pt), 1 = sum(sq*bce), 2 = sum(dil2*bce), 3 = sum(ero2*bce)
    dots_d = nc.dram_tensor("dots", [4, P, SW], f32, kind="ExternalOutput").ap()

    with tile.TileContext(nc) as tc:
        with (
            tc.tile_pool(name="io", bufs=2) as io,
            tc.tile_pool(name="ew", bufs=2) as ew,
            tc.tile_pool(name="mo", bufs=1) as mo,
            tc.tile_pool(name="st", bufs=1) as st,
            tc.tile_pool(name="ps", bufs=1, space="PSUM") as ps,
        ):
            acc = st.tile([P, 16], f32)
            nc.vector.memset(acc[:], 0.0)
            dots = [ps.tile([P, SW], f32, tag=f"dot{i}", name=f"dot{i}")
                    for i in range(4)]

            for s in range(STRIPS):
                c0 = s * SW
                # ---- loads ----
                xs = io.tile([P, ROWS_PB, SW], f32, tag="xs")
                ts = io.tile([P, ROWS_PB, SW + 4], f32, tag="ts")
                nc.sync.dma_start(xs[:], x_d[:, :, c0:c0 + SW])
                lo = max(c0 - 2, 0)
                hi = min(c0 + SW + 2, W)
                dst0 = lo - (c0 - 2)          # 2 for s==0 else 0
                nc.sync.dma_start(ts[:, :, dst0:dst0 + (hi - lo)],
                                  t_d[:, :, lo:hi])

                # ---- zh = (t - 0.5) * x  (bf16) ----
                zh = ew.tile([P, ROWS_PB, SW], bf16, tag="zh", bufs=1)
                nc.vector.scalar_tensor_tensor(
                    out=zh[:], in0=ts[:, :, 2:2 + SW], scalar=0.5,
                    in1=xs[:], op0=Alu.subtract, op1=Alu.mult)

                # ---- mask tensor with halos: mP [P, 36, SW+4] bf16 ----
                mP = mo.tile([P, ROWS_PB + 4, SW + 4], bf16, tag="mP")
                # center conversion, accumulating sum(t) over core columns
                nc.vector.tensor_scalar(
                    out=mP[:, 2:2 + ROWS_PB, 2:2 + SW],
                    in0=ts[:, :, 2:2 + SW],
                    scalar1=1.0, scalar2=0.0, op0=Alu.mult, op1=Alu.add,
                    accum_out=acc[:, s:s + 1])
                # column halos
                if s == 0:
                    nc.vector.memset(mP[:, 2:2 + ROWS_PB, 0:2], 0.0)
                else:
                    nc.vector.tensor_copy(mP[:, 2:2 + ROWS_PB, 0:2],
                                          ts[:, :, 0:2])
                if s == STRIPS - 1:
                    nc.vector.memset(mP[:, 2:2 + ROWS_PB, SW + 2:SW + 4], 0.0)
                else:
                    nc.vector.tensor_copy(mP[:, 2:2 + ROWS_PB, SW + 2:SW + 4],
                                          ts[:, :, SW + 2:SW + 4])
                # zero all halo rows, then fill intra-image halos from the
                # neighbouring partition (SBUF->SBUF DMA); image-boundary
                # partitions keep the zeros (scipy border_value=0).
                nc.vector.memset(mP[:, 0:2, :], 0.0)
                nc.vector.memset(mP[:, 34:36, :], 0.0)
                for i in range(IMG_PER_CORE):
                    b = 16 * i
                    nc.sync.dma_start(mP[b + 1:b + 16, 0:2, :],
                                      mP[b:b + 15, 32:34, :])
                    nc.sync.dma_start(mP[b:b + 15, 34:36, :],
                                      mP[b + 1:b + 16, 2:4, :])

                # ---- ACT passes (all funcs live in one ACT table:
                #      natural_log_exp_and_others) ----
                q = ew.tile([P, ROWS_PB, SW], bf16, tag="q", bufs=1)
                pt = ew.tile([P, ROWS_PB, SW], bf16, tag="pt")
                bce = ew.tile([P, ROWS_PB, SW], bf16, tag="bce")
                sq = ew.tile([P, ROWS_PB, SW], bf16, tag="sq")
                # q = exp(-2*zh) = exp(-z)
                nc.scalar.activation(q[:], zh[:], Act.Exp, scale=-2.0)
                # bce = ln(1 + q) = softplus(-z)
                nc.scalar.activation(bce[:], q[:], Act.Ln, bias=1.0,
                                     accum_out=acc[:, 8 + s:9 + s])
                # pt = exp(-bce) = sigmoid(z)
                nc.scalar.activation(pt[:], bce[:], Act.Exp, scale=-1.0,
                                     accum_out=acc[:, 4 + s:5 + s])
                # sq = (1 - pt)^2
                nc.scalar.activation(sq[:], pt[:], Act.Square, bias=1.0,
                                     scale=-1.0)

                # ---- morphology: 13-point diamond stencil sum ----
                R = ROWS_PB
                A = mo.tile([P, R, SW + 4], bf16, tag="A")
                nc.vector.tensor_add(A[:], mP[:, 1:1 + R, :], mP[:, 3:3 + R, :])
                B = mo.tile([P, R, SW], bf16, tag="B")
                nc.vector.tensor_add(B[:], mP[:, 0:R, 2:2 + SW],
                                     mP[:, 4:4 + R, 2:2 + SW])
                u = mo.tile([P, R, SW], bf16, tag="u")
                nc.vector.tensor_add(u[:], mP[:, 2:2 + R, 0:SW],
                                     mP[:, 2:2 + R, 4:4 + SW])
                v = mo.tile([P, R, SW], bf16, tag="v")
                nc.vector.tensor_add(v[:], mP[:, 2:2 + R, 1:1 + SW],
                                     mP[:, 2:2 + R, 3:3 + SW])
                p1 = mo.tile([P, R, SW], bf16, tag="p1")
                nc.vector.tensor_add(p1[:], A[:, :, 1:1 + SW], A[:, :, 3:3 + SW])
                P3 = mo.tile([P, R, SW], bf16, tag="P3")
                nc.vector.tensor_add(P3[:], p1[:], A[:, :, 2:2 + SW])
                w7 = mo.tile([P, R, SW], bf16, tag="w7")
                nc.vector.tensor_add(w7[:], B[:], u[:])
                w8 = mo.tile([P, R, SW], bf16, tag="u")   # reuse u slot
                nc.vector.scalar_tensor_tensor(
                    out=w8[:], in0=v[:], scalar=2.0, in1=w7[:],
                    op0=Alu.mult, op1=Alu.add)
                w9 = mo.tile([P, R, SW], bf16, tag="B")   # reuse B slot
                nc.vector.scalar_tensor_tensor(
                    out=w9[:], in0=mP[:, 2:2 + R, 2:2 + SW], scalar=5.0,
                    in1=w8[:], op0=Alu.mult, op1=Alu.add)
                wsum = mo.tile([P, R, SW], bf16, tag="w7")  # reuse w7 slot
                nc.vector.scalar_tensor_tensor(
                    out=wsum[:], in0=P3[:], scalar=2.0, in1=w9[:],
                    op0=Alu.mult, op1=Alu.add)
                dil = mo.tile([P, R, SW], bf16, tag="v")    # reuse v slot
                nc.vector.tensor_scalar(
                    out=dil[:], in0=wsum[:], scalar1=0.5, scalar2=None,
                    op0=Alu.is_ge)
                ero = mo.tile([P, R, SW], bf16, tag="p1")   # reuse p1 slot
                nc.vector.tensor_scalar(
                    out=ero[:], in0=wsum[:], scalar1=24.5, scalar2=None,
                    op0=Alu.is_ge)

                # ---- dot products on TensorE (PSUM diag-accumulate) ----
                first = s == 0
                last = s == STRIPS - 1
                for k in range(ROWS_PB):
                    fk = first and k == 0
                    lk = last and k == ROWS_PB - 1
                    m_k = mP[:, 2 + k, 2:2 + SW]
                    nc.tensor.matmul(dots[0][:], pt[:, k, :], m_k,
                                     start=fk, stop=lk)
                    nc.tensor.matmul(dots[1][:], sq[:, k, :], bce[:, k, :],
                                     start=fk, stop=lk)
                    nc.tensor.matmul(dots[2][:], dil[:, k, :], bce[:, k, :],
                                     start=fk, stop=lk)
                    nc.tensor.matmul(dots[3][:], ero[:, k, :], bce[:, k, :],
                                     start=fk, stop=lk)

            # ---- write outputs ----
            nc.sync.dma_start(acc_d[:], acc[:])
            for i in range(4):
                ob = st.tile([P, SW], f32, tag=f"ob{i}")
                nc.scalar.copy(ob[:], dots[i][:])
                nc.sync.dma_start(dots_d[i], ob[:])

    nc.compile()
    return nc


def _get_nc():
    if "nc" not in _CACHE:
        _CACHE["nc"] = _build()
    return _CACHE["nc"]


def kernel(inputs: np.ndarray, targets: np.ndarray) -> np.ndarray:
    from concourse.bass_utils import run_bass_kernel_spmd

    nc = _get_nc()

    x = np.asarray(inputs, dtype=np.float32).reshape(64, H, W)
    t = np.asarray(targets, dtype=np.float32).reshape(64, H, W)

    in_maps = []
    for c in range(N_CORES):
        xc = x[c * IMG_PER_CORE:(c + 1) * IMG_PER_CORE]
        tc_ = t[c * IMG_PER_CORE:(c + 1) * IMG_PER_CORE]
        xc = np.ascontiguousarray(xc.reshape(P, ROWS_PB, W))
        tc_ = np.ascontiguousarray(tc_.reshape(P, ROWS_PB, W))
        in_maps.append({"x": xc, "t": tc_})

    import os
    trace = bool(os.environ.get("BASS_TRACE_KERNEL"))
    res = run_bass_kernel_spmd(nc, in_maps, core_ids=list(range(N_CORES)),
                               trace=trace)
    _CACHE["exec_time_ns"] = res.exec_time_ns

    sum_t = sum_pt = sum_bce = 0.0
    d_tpt = d_focal = d_dil = d_ero = 0.0
    for c in range(N_CORES):
        acc = res.results[c]["acc"].astype(np.float64)
        dots = res.results[c]["dots"].astype(np.float64)
        sum_t += acc[:, 0:4].sum()
        sum_pt += acc[:, 4:8].sum()
        sum_bce += acc[:, 8:12].sum()
        d_tpt += np.trace(dots[0])
        d_focal += np.trace(dots[1])
        d_dil += np.trace(dots[2])
        d_ero += np.trace(dots[3])

    n = float(N_TOTAL)
    focal_loss = 0.25 * d_focal / n
    sum_p = n - sum_t - sum_pt + 2.0 * d_tpt
    dice = (2.0 * d_tpt + 1e-6) / (sum_p + sum_t + 1e-6)
    dice_loss = 1.0 - dice
    boundary_loss = (sum_bce + 5.0 * (d_dil - d_ero)) / n
    loss = 0.3 * focal_loss + 0.4 * dice_loss + 0.3 * boundary_loss
    return np.float32(loss)

